# revision 35
# baseline (speedup 1.0000x reference)
"""Trainium2 Bass kernel for nn_CvxDifflayer (batched PDHG LP solver).

Math (per batch row b):
    u_{k+1} = clip(u_k - tau*(q + y_k @ K), 0, 1)
    ubar    = 2*u_{k+1} - u_k
    y_{k+1} = relu(y_k + sigma*(ubar @ K.T - h))
    out z   = u_300[:, V:]  reshaped (12, 12)

Device reformulation (exact, per 64-batch shard):
    G_k  = tau*q + tau*(y_k @ K)        MM1: lhsT = Y feat-major, rhs = tau*K
    u    = clip(pres_k - G'_k)          where pres_k = u_k - tau*q, G' = tau*(yK)
    pres = u - tau*q                    (GPSIMD, off critical path)
    P_k  = sigma*(u @ K.T) - sigma*h    MM2: lhsT = u feat-major (PE transposes),
                                        h folded via constant ones-row in lhsT
    y    = relu(y + 2*P_k - P_{k-1})    (P_{-1} = -sigma*h since u_0 = 0)

Layouts per core (batch shard Bs=64):
    U     [128, 578]  batch-major fold: row 64*hf+b, col j = feature 578*hf+j
    UFM   [128, 640]  feat-major: chunk c cols [128c:128c+128); within chunk,
                      col 64*h+b = batch b of fold-half h; rows = fold-col
                      128c+i; chunk 4 rows 0:66 (+ row 66 = ones for h-fold)
    YBM   [64, 288]   batch-major y
    YFM   [128, 192]  feat-major y: block ci cols [64ci:64ci+64) = batch,
                      rows = y-row 128ci+i
"""

import sys

for _p in ("/opt/trn_rl_repo", "/opt/pypackages"):
    if _p not in sys.path:
        sys.path.insert(0, _p)

import numpy as np

N_GRID = 12
N = 144          # nodes
V = 1012         # directed edges
F = V + N        # 1156 primal vars
YR = 2 * N       # 288 dual vars
B = 512
BS = 64          # batch per core
N_CORES = 8
ITERS = 300
FPAD = 1280      # MM1 free dim padded so all chunks >= 256 (fp32r rate)
FM_CHUNKS = 10   # ceil(1156/128)
LAST_CW = F - 9 * 128      # 4


def _tables():
    offs = [(p, q) for p in (-1, 0, 1) for q in (-1, 0, 1) if (p, q) != (0, 0)]
    es = []
    for i in range(N_GRID):
        for j in range(N_GRID):
            for p, q in offs:
                ii, jj = i + p, j + q
                if 0 <= ii < N_GRID and 0 <= jj < N_GRID:
                    es.append((i * N_GRID + j, ii * N_GRID + jj))

    def nb(node):
        return (node // N_GRID) // 4

    keys = [(min(nb(s), nb(t)), max(nb(s), nb(t)), s) for (s, t) in es]
    order = sorted(range(V), key=lambda e: keys[e])
    perm_e = np.array(order, dtype=np.int64)
    segs = []
    st, cur = 0, keys[order[0]][:2]
    for j in range(1, V + 1):
        if j == V or keys[order[j]][:2] != cur:
            segs.append((cur[0], cur[1], st, j))
            if j < V:
                cur, st = keys[order[j]][:2], j
    ss = []
    for (b1, b2, s0, s1) in segs:
        if s0 < 512 < s1:
            ss += [(b1, b2, s0, 512), (b1, b2, 512, s1)]
        else:
            ss.append((b1, b2, s0, s1))
    mm1 = []
    for bank in (0, 1):
        for bd in range(3):
            for (b1, b2, s0, s1) in ss:
                if bd in (b1, b2) and ((s1 <= 512) == (bank == 0)):
                    mm1.append((bd, s0, s1, bd == b1))
    y_perm = np.empty(YR, dtype=np.int64)
    y_perm[0::2] = np.arange(N)
    y_perm[1::2] = N + np.arange(N)
    chunks = []
    for c0 in range(0, V, 128):
        c1 = min(c0 + 128, V)
        nodes = set()
        for j in range(c0, c1):
            s, t = es[perm_e[j]]
            nodes.add(s)
            nodes.add(t)
        chunks.append((c0, c1, 2 * min(nodes), 2 * max(nodes) + 2))
    return perm_e, y_perm, mm1, chunks


PERM_E, Y_PERM, MM1_TAB, MM2_CHUNKS = _tables()
BAND_COL0 = [0, 292, 652]
KA1_OFF = [0, 360, 788]
KS2_OFF = np.cumsum(
    [0] + [hi - lo for (_, _, lo, hi) in MM2_CHUNKS]).tolist()


def _mm1_rhs_off(bd, c0):
    if bd == 1 and c0 >= 512:
        return 580 + (c0 - 512)
    if bd == 2 and c0 >= 720:
        return 856 + (c0 - 720)
    return KA1_OFF[bd] + (c0 - BAND_COL0[bd])


def _build_constants(A, A_pos, b):
    K = np.zeros((YR, F), np.float32)
    K[:N, :V] = A
    K[N:, :V] = A_pos
    K[N:, V:] = -np.eye(N, dtype=np.float32)
    h = np.concatenate([b.astype(np.float32), np.zeros(N, np.float32)])
    Kn = np.float32(np.sqrt(np.abs(K).sum(0).max() * np.abs(K).sum(1).max()))
    tau = np.float32(0.9) / Kn
    return K, h, tau


def _host_tiles(K, h, tau):
    """Banded constant SBUF images (edge/y permuted) shared by all cores."""
    sigma = tau
    Kx = K[:, :V][np.ix_(Y_PERM, PERM_E)]        # (288, 1012) permuted
    hp = h[Y_PERM]
    tKx = (tau * Kx).astype(np.float32)
    sKx = (sigma * Kx).astype(np.float32)

    # KA1: MM1 rhs band blocks: band0 x[0:360) @0, band1 x[292:512) @360,
    # band1 x[512:720) @580, band2 x[652:720) @788, band2 x[720:1012) @856
    ka1 = np.zeros((128, KA1_W), np.float32)
    for bd, r0, c0, w, off in ((0, 0, 0, 360, 0), (1, 96, 292, 220, 360),
                               (1, 96, 512, 208, 580), (2, 192, 652, 68, 788),
                               (2, 192, 720, 292, 856)):
        ka1[0:96, off:off + w] = tKx[r0:r0 + 96, c0:c0 + w]

    # KS2: MM2 span blocks + h-row at [KS2_H:KS2_H+288) row 0
    ks2 = np.zeros((128, KS2_W), np.float32)
    for ci, (c0, c1, ylo, yhi) in enumerate(MM2_CHUNKS):
        ks2[0:c1 - c0, KS2_OFF[ci]:KS2_OFF[ci + 1]] = sKx[ylo:yhi, c0:c1].T
    ks2[0, KS2_H:KS2_H + YR] = -sigma * hp
    return ka1, ks2


def _per_core_tiles(w_shard, tau):
    """tq for one 64-row batch shard; w_shard (64, 144)."""
    tq = np.zeros((64, F), np.float32)
    tq[:, V:] = tau * w_shard
    return tq


# constsr layout (f32r, read-only): ka1 | ks2
KA1_W = 1148
KS2_H = KS2_OFF[-1]
KS2_W = KS2_H + YR
C_KA1 = 0
C_KS2 = C_KA1 + KA1_W
CR_W = C_KS2 + KS2_W
# constsf layout (f32, read-only): tq | yp0  (yp0 = +sigma*h)
C_TQ = 0
C_YP = C_TQ + F
CF_W = C_YP + YR

FA = 512          # feature split: A = 0:512, B = 512:1156
FB = F - FA       # 644


def _pack_consts(ka1, ks2, tq, yp0):
    cr = np.zeros((128, CR_W), np.float32)
    cr[:, C_KA1:C_KS2] = ka1
    cr[:, C_KS2:CR_W] = ks2
    cf = np.zeros((64, CF_W), np.float32)
    cf[:, C_TQ:C_YP] = tq
    cf[:, C_YP:CF_W] = yp0
    return cr, cf


TAUS = np.zeros(1, np.float32)


def _build_bass():
    from concourse import bass, mybir
    from concourse.tile import TileContext
    from concourse.tile_rust import add_dep_helper
    from concourse.mybir import AluOpType as op

    f32 = mybir.dt.float32
    f32r = mybir.dt.float32r

    nc = bass.Bass()
    d_cr = nc.dram_tensor("constsr", (128, CR_W), f32r, kind="ExternalInput")
    d_cf = nc.dram_tensor("constsf", (64, CF_W), f32, kind="ExternalInput")
    d_z = nc.dram_tensor("z", (64, N), f32, kind="ExternalOutput")

    with TileContext(nc) as tc:
        with (
            tc.tile_pool(name="state", bufs=1) as sp,
            tc.tile_pool(name="psA", bufs=1, space="PSUM") as psA,
            tc.tile_pool(name="psB", bufs=1, space="PSUM") as psB,
            tc.tile_pool(name="psP", bufs=1, space="PSUM") as psP,
            tc.tile_pool(name="psT0", bufs=1, space="PSUM") as psT0,
            tc.tile_pool(name="psT1", bufs=1, space="PSUM") as psT1,
            tc.tile_pool(name="psTY", bufs=1, space="PSUM") as psTY,
        ):
            CONSTR = sp.tile([128, CR_W], f32r)
            CONSTF = sp.tile([64, CF_W], f32)
            KA1 = CONSTR[:, C_KA1:C_KS2]
            KS2 = CONSTR[:, C_KS2:CR_W]
            TQ_A = CONSTF[:, C_TQ:C_TQ + FA]
            TQ_B = CONSTF[:, C_TQ + FA:C_YP]
            U_A = sp.tile([64, FA], f32)
            U_B = sp.tile([64, FB], f32)
            TMP_A = sp.tile([64, FA], f32)
            TMP_B = sp.tile([64, FB], f32)
            PRES_A = sp.tile([64, FA], f32)
            PRES_B = sp.tile([64, FB], f32)
            YP = sp.tile([64, YR], f32)
            YBM = sp.tile([64, YR], f32)
            T3 = sp.tile([64, YR], f32)
            ZPREV = sp.tile([64, N], f32)
            # per-engine scratch tiles (separate so absorber ops never
            # create cross-engine tile deps)
            SCRD = sp.tile([32, 8], f32)
            SCRA = sp.tile([32, 12], f32)
            SCRP = sp.tile([32, 8], f32)
            ONES32 = sp.tile([32, 64], f32)
            ZER128 = sp.tile([128, 192], f32)
            # feat-major u in two wide tiles; ones-row for the h-fold at
            # row LAST_CW of the chunk-9 column block of UFM1
            UFM0 = sp.tile([128, 256], f32r)   # chunks 0..3
            UFM45 = sp.tile([128, 128], f32r)  # chunks 4,5
            UFM67 = sp.tile([128, 128], f32r)  # chunks 6,7
            UFM8 = sp.tile([128, 64], f32r)    # chunk 8
            UFM9 = sp.tile([32, 64], f32r)     # chunk 9 + ones row
            YFM = sp.tile([96, 192], f32r)
            IDENT = sp.tile([128, 128], f32)

            dma1 = nc.sync.dma_start(CONSTR[:, :], d_cr[:, :])
            dma2 = nc.sync.dma_start(CONSTF[:, :], d_cf[:, :])

            pool_insts = [
                nc.gpsimd.memset(IDENT[:, :], 0.0),
                nc.gpsimd.affine_select(
                    out=IDENT[:, :], in_=IDENT[:, :],
                    compare_op=mybir.AluOpType.not_equal, fill=1.0, base=0,
                    pattern=[[-1, 128]], channel_multiplier=1),
            ]
            dve_insts = [
                nc.vector.memset(U_A[:, :], 0.0),
                nc.vector.memset(U_B[:, :], 0.0),
                nc.vector.memset(ONES32[:, :], 1.0),
                nc.vector.memset(ZER128[:, :], 0.0),
                nc.vector.memset(YBM[:, :], 0.0),
            ]

            G_A = psA.tile([64, FA], f32)
            G_B = psB.tile([64, 768], f32)
            P = psP.tile([64, YR], f32)
            TPX = psT0.tile([128, 256], f32)   # chunks 0..3, then 8..9
            TP45 = psT1.tile([128, 128], f32)
            TP67 = psT1.tile([128, 128], f32, tag="tp67")
            TY = psTY.tile([128, 192], f32)

            # This target allows only ONE sem wait per instruction. Tile's
            # wait elision relies on per-engine program order, which the
            # scheduler may permute. So: (a) pin every engine's stream to
            # emission order with no_sync edges, (b) warm each engine with
            # ops that absorb foreign sems one at a time, (c) per iteration,
            # absorber ops pick up semaphores so every real instruction
            # needs at most one new wait.
            prev = {}

            def chain(eng, inst, *sync_deps):
                for d in sync_deps:
                    add_dep_helper(inst.ins, d.ins, True, "warm")
                if eng in prev:
                    add_dep_helper(inst.ins, prev[eng].ins, False, "order")
                prev[eng] = inst
                return inst

            def pe(inst, *d):
                return chain("pe", inst, *d)

            def dve(inst, *d):
                return chain("dve", inst, *d)

            def act(inst, *d):
                return chain("act", inst, *d)

            def pool(inst, *d):
                return chain("pool", inst, *d)

            # engine warmups: absorb one foreign semaphore per instruction
            dve(nc.vector.tensor_copy(SCRD[0:32, 0:4], CONSTF[0:32, 0:4]),
                dma2)
            dve(nc.vector.tensor_scalar_mul(PRES_A[:, :], TQ_A, -1.0))
            dve(nc.vector.tensor_scalar_mul(PRES_B[:, :], TQ_B, -1.0))
            dve(nc.vector.tensor_copy(YP[:, :], CONSTF[:, C_YP:CF_W]))
            pool(nc.gpsimd.tensor_copy(SCRP[0:32, 4:8], CONSTF[0:32, 4:8]),
                 dma2)
            act(nc.scalar.copy(SCRA[0:32, 8:12], IDENT[0:32, 0:4]),
                *pool_insts)
            act(nc.scalar.copy(UFM9[0:32, 0:64], ONES32[:, :]),
                *dve_insts)
            act(nc.scalar.copy(YFM[0:96, 0:192], ZER128[0:96, 0:192]))
            pe(nc.tensor.transpose(G_A[0:64, 0:64], IDENT[0:64, 0:64],
                                   IDENT[0:64, 0:64]),
               *pool_insts)
            pe(nc.tensor.transpose(G_A[0:64, 0:64], U_A[:, 0:64],
                                   IDENT[0:64, 0:64]),
               *dve_insts)
            pe(nc.tensor.matmul(G_A[0:64, 0:64], KS2[0:128, 0:64],
                                KA1[0:128, 0:64], start=True, stop=True))

            for _it in range(ITERS):
                # ACT absorbers: a1 waits on the last ACT op of the previous
                # iteration (the YFM copy); a2 waits on a1's completion.
                act(nc.scalar.copy(SCRA[0:32, 0:4], YFM[0:32, 128:132]))
                act(nc.scalar.copy(SCRA[0:32, 4:8], SCRA[0:32, 0:4]))

                # ---- MM1: G = tau*(y @ Kx), banded segments; closed
                # psum groups per segment (multi-group deps merge on the
                # PE sem for the DVE readers) ----
                from collections import Counter as _C
                segw = _C((c0, c1) for (_, c0, c1, _) in MM1_TAB)
                seen = _C()
                for (bd, c0, c1, first) in MM1_TAB:
                    seen[(c0, c1)] += 1
                    last = seen[(c0, c1)] == segw[(c0, c1)]
                    out = (G_A[:, c0:c1] if c1 <= 512
                           else G_B[:, c0 - 512:c1 - 512])
                    off = _mm1_rhs_off(bd, c0)
                    pe(nc.tensor.matmul(
                        out, YFM[0:96, 64 * bd:64 * bd + 64],
                        KA1[0:96, off:off + (c1 - c0)],
                        start=first, stop=last, skip_group_check=True))

                # ---- u update, A then B (DVE), pres on GPSIMD ----
                dve(nc.vector.tensor_copy(SCRD[0:32, 0:2], PRES_A[0:32, 0:2]))
                dve(nc.vector.scalar_tensor_tensor(
                    TMP_A[:, :], G_A[:, :], -1.0, PRES_A[:, :],
                    op.mult, op.add))
                dve(nc.vector.tensor_scalar(
                    U_A[:, :], TMP_A[:, :], 0.0, 1.0, op.max, op.min))
                dve(nc.vector.tensor_copy(SCRD[0:32, 2:4], PRES_B[0:32, 0:2]))
                dve(nc.vector.scalar_tensor_tensor(
                    TMP_B[:, 0:500], G_B[:, 0:500], -1.0, PRES_B[:, 0:500],
                    op.mult, op.add))
                dve(nc.vector.scalar_tensor_tensor(
                    TMP_B[:, 500:FB], YBM[:, 1::2], float(TAUS[0]),
                    PRES_B[:, 500:FB], op.mult, op.add))
                dve(nc.vector.tensor_scalar(
                    U_B[:, :], TMP_B[:, :], 0.0, 1.0, op.max, op.min))
                pool(nc.gpsimd.tensor_copy(SCRP[0:32, 2:4], SCRP[0:32, 0:2]))
                pool(nc.gpsimd.tensor_sub(PRES_A[:, :], U_A[:, :], TQ_A))
                pool(nc.gpsimd.tensor_sub(PRES_B[:, :], U_B[:, :], TQ_B))
                pool(nc.gpsimd.tensor_copy(SCRP[0:32, 0:2], PRES_B[0:32, 0:2]))

                # ---- transpose u to feat-major; MM2 accumulates P ----
                for c in range(4):        # chunks 0..3 from U_A
                    pe(nc.tensor.transpose(
                        TPX[:, 64 * c:64 * c + 64],
                        U_A[:, 128 * c:128 * c + 128], IDENT[0:64, 0:64]))
                act(nc.scalar.copy(UFM0[:, :], TPX[:, :]))
                # absorber: pick up ufm0's completion so the later TPX
                # read-read serializer deps (chunks 8/9) are pre-covered
                act(nc.scalar.copy(SCRA[0:32, 8:12], UFM0[0:32, 0:4]))
                pe(nc.tensor.matmul(
                    P[:, :], UFM9[0:1, 0:64], KS2[0:1, KS2_H:KS2_H + YR],
                    start=True, stop=True, skip_group_check=True))
                for c in range(4):
                    c0, c1, ylo, yhi = MM2_CHUNKS[c]
                    pe(nc.tensor.matmul(
                        P[:, ylo:yhi], UFM0[0:128, 64 * c:64 * c + 64],
                        KS2[0:128, KS2_OFF[c]:KS2_OFF[c + 1]],
                        start=False, stop=True,
                        skip_group_check=True))
                # chunks 4..9 from U_B, grouped (2 transposes -> copy ->
                # 2 matmuls) so MM2 starts as soon as each pair lands
                def t1(c, dst, col):
                    cw = 128 if c < 9 else LAST_CW
                    pe(nc.tensor.transpose(
                        dst[0:cw, col:col + 64],
                        U_B[:, 128 * (c - 4):128 * (c - 4) + cw],
                        IDENT[0:64, 0:64]))

                def mm2(c, tile, col, stop=False):
                    rows = 128 if c < 7 else 116
                    c0, c1, ylo, yhi = MM2_CHUNKS[c]
                    return pe(nc.tensor.matmul(
                        P[:, ylo:yhi], tile[0:rows, col:col + 64],
                        KS2[0:rows, KS2_OFF[c]:KS2_OFF[c + 1]],
                        start=False, stop=True,
                        skip_group_check=True))

                t1(4, TP45, 0)
                t1(5, TP45, 64)
                t1(6, TP67, 0)
                t1(7, TP67, 64)
                act(nc.scalar.copy(UFM45[:, :], TP45[:, :]))
                act(nc.scalar.copy(UFM67[:, :], TP67[:, :]))
                mm2(4, UFM45, 0)
                mm2(5, UFM45, 64)
                mm2(6, UFM67, 0)
                mm2(7, UFM67, 64)

                # ---- y update pipelined into MM2: band b of P is
                # complete after chunk {2,5,7}; T3/fix/relu/transpose per
                # band while later MM2 chunks still run ----
                for ci in range(3):
                    c0 = 96 * ci
                    dve(nc.vector.scalar_tensor_tensor(
                        T3[:, c0:c0 + 96], P[:, c0:c0 + 96], 2.0,
                        YP[:, c0:c0 + 96], op.mult, op.add))
                    dve(nc.vector.scalar_tensor_tensor(
                        T3[:, c0 + 1:c0 + 96:2], U_B[:, 500 + 48 * ci:
                        500 + 48 * ci + 48], -2.0 * float(TAUS[0]),
                        T3[:, c0 + 1:c0 + 96:2], op.mult, op.add))
                    dve(nc.vector.tensor_scalar_max(
                        YBM[:, c0:c0 + 96], T3[:, c0:c0 + 96], 0.0))
                    pe(nc.tensor.transpose(
                        TY[0:96, 64 * ci:64 * ci + 64],
                        YBM[:, c0:c0 + 96],
                        IDENT[0:64, 0:64]))
                act(nc.scalar.copy(YFM[0:96, 0:64], TY[0:96, 0:64]))
                act(nc.scalar.copy(YFM[0:96, 64:192], TY[0:96, 64:192]))
                # off-critical: YP for next iter (reads P psum, so DVE)
                dve(nc.vector.scalar_tensor_tensor(
                    YP[:, :], P[:, :], -1.0, YBM[:, :], op.mult, op.add))
                dve(nc.vector.scalar_tensor_tensor(
                    YP[:, 1::2], U_B[:, 500:FB], float(TAUS[0]),
                    YP[:, 1::2], op.mult, op.add))


            zdma = nc.sync.dma_start(d_z[:, :], U_B[:, FB - N:FB])
            # tail fence: the framework drain waits on every proc, but the
            # ISA allows one wait per instruction — absorb them one at a
            # time with SP nops so the drain's own waits are elided.
            for d in (dma1, dma2, prev["pool"], prev["act"], prev["pe"],
                      prev["dve"], zdma):
                nn = nc.sync.nop()
                add_dep_helper(nn.ins, d.ins, True, "tail fence")
    return nc


LAST_RESULT = None


def kernel(weights, A, A_pos, b, _trace=False):
    weights = np.asarray(weights, np.float32)
    A = np.asarray(A, np.float32)
    A_pos = np.asarray(A_pos, np.float32)
    b = np.asarray(b, np.float32)

    K, h, tau = _build_constants(A, A_pos, b)
    TAUS[0] = tau
    ka1, ks2 = _host_tiles(K, h, tau)
    yp0 = np.broadcast_to((tau * h)[Y_PERM], (64, YR)).astype(
        np.float32).copy()

    nc = _build_bass()

    in_maps = []
    for core in range(N_CORES):
        w_shard = weights[core * BS:(core + 1) * BS].reshape(BS, N)
        tq = _per_core_tiles(w_shard, tau)
        cr, cf = _pack_consts(ka1, ks2, tq, yp0)
        in_maps.append({"constsr": cr, "constsf": cf})

    from concourse.bass_utils import run_bass_kernel_spmd
    res = run_bass_kernel_spmd(nc, in_maps, core_ids=list(range(N_CORES)),
                               trace=_trace)
    global LAST_RESULT
    LAST_RESULT = res
    outs = [np.asarray(res.results[c]["z"]) for c in range(N_CORES)]
    z = np.concatenate(outs, axis=0).reshape(B, N_GRID, N_GRID)
    return z.astype(np.float32)


if __name__ == "__main__":
    TAUS[0] = 0.1
    rng = np.random.default_rng(0)
    w = rng.random((B, N_GRID, N_GRID), np.float32)
    # smoke build only
    _build_bass()
    print("bass build OK")



# revision 36
# speedup vs baseline: 1.0949x; 1.0949x over previous
"""Trainium2 Bass kernel for nn_CvxDifflayer (batched PDHG LP solver).

Math (per batch row b):
    u_{k+1} = clip(u_k - tau*(q + y_k @ K), 0, 1)
    ubar    = 2*u_{k+1} - u_k
    y_{k+1} = relu(y_k + sigma*(ubar @ K.T - h))
    out z   = u_300[:, V:]  reshaped (12, 12)

Device reformulation (exact, per 64-batch shard):
    G_k  = tau*q + tau*(y_k @ K)        MM1: lhsT = Y feat-major, rhs = tau*K
    u    = clip(pres_k - G'_k)          where pres_k = u_k - tau*q, G' = tau*(yK)
    pres = u - tau*q                    (GPSIMD, off critical path)
    P_k  = sigma*(u @ K.T) - sigma*h    MM2: lhsT = u feat-major (PE transposes),
                                        h folded via constant ones-row in lhsT
    y    = relu(y + 2*P_k - P_{k-1})    (P_{-1} = -sigma*h since u_0 = 0)

Layouts per core (batch shard Bs=64):
    U     [128, 578]  batch-major fold: row 64*hf+b, col j = feature 578*hf+j
    UFM   [128, 640]  feat-major: chunk c cols [128c:128c+128); within chunk,
                      col 64*h+b = batch b of fold-half h; rows = fold-col
                      128c+i; chunk 4 rows 0:66 (+ row 66 = ones for h-fold)
    YBM   [64, 288]   batch-major y
    YFM   [128, 192]  feat-major y: block ci cols [64ci:64ci+64) = batch,
                      rows = y-row 128ci+i
"""

import sys

for _p in ("/opt/trn_rl_repo", "/opt/pypackages"):
    if _p not in sys.path:
        sys.path.insert(0, _p)

import numpy as np

N_GRID = 12
N = 144          # nodes
V = 1012         # directed edges
F = V + N        # 1156 primal vars
YR = 2 * N       # 288 dual vars
B = 512
BS = 64          # batch per core
N_CORES = 8
ITERS = 300
FPAD = 1280      # MM1 free dim padded so all chunks >= 256 (fp32r rate)
FM_CHUNKS = 10   # ceil(1156/128)
LAST_CW = F - 9 * 128      # 4


def _tables():
    offs = [(p, q) for p in (-1, 0, 1) for q in (-1, 0, 1) if (p, q) != (0, 0)]
    es = []
    for i in range(N_GRID):
        for j in range(N_GRID):
            for p, q in offs:
                ii, jj = i + p, j + q
                if 0 <= ii < N_GRID and 0 <= jj < N_GRID:
                    es.append((i * N_GRID + j, ii * N_GRID + jj))

    def nb(node):
        return (node // N_GRID) // 4

    keys = [(min(nb(s), nb(t)), max(nb(s), nb(t)), s) for (s, t) in es]
    order = sorted(range(V), key=lambda e: keys[e])
    perm_e = np.array(order, dtype=np.int64)
    segs = []
    st, cur = 0, keys[order[0]][:2]
    for j in range(1, V + 1):
        if j == V or keys[order[j]][:2] != cur:
            segs.append((cur[0], cur[1], st, j))
            if j < V:
                cur, st = keys[order[j]][:2], j
    ss = []
    for (b1, b2, s0, s1) in segs:
        if s0 < 512 < s1:
            ss += [(b1, b2, s0, 512), (b1, b2, 512, s1)]
        else:
            ss.append((b1, b2, s0, s1))
    mm1 = []
    for bank in (0, 1):
        for bd in range(3):
            for (b1, b2, s0, s1) in ss:
                if bd in (b1, b2) and ((s1 <= 512) == (bank == 0)):
                    mm1.append((bd, s0, s1, bd == b1))
    y_perm = np.empty(YR, dtype=np.int64)
    y_perm[0::2] = np.arange(N)
    y_perm[1::2] = N + np.arange(N)
    chunks = []
    for c0 in range(0, V, 128):
        c1 = min(c0 + 128, V)
        nodes = set()
        for j in range(c0, c1):
            s, t = es[perm_e[j]]
            nodes.add(s)
            nodes.add(t)
        chunks.append((c0, c1, 2 * min(nodes), 2 * max(nodes) + 2))
    return perm_e, y_perm, mm1, chunks


PERM_E, Y_PERM, MM1_TAB, MM2_CHUNKS = _tables()
BAND_COL0 = [0, 292, 652]
KA1_OFF = [0, 360, 788]
KS2_OFF = np.cumsum(
    [0] + [hi - lo for (_, _, lo, hi) in MM2_CHUNKS]).tolist()


def _mm1_rhs_off(bd, c0):
    if bd == 1 and c0 >= 512:
        return 580 + (c0 - 512)
    if bd == 2 and c0 >= 720:
        return 856 + (c0 - 720)
    return KA1_OFF[bd] + (c0 - BAND_COL0[bd])


def _build_constants(A, A_pos, b):
    K = np.zeros((YR, F), np.float32)
    K[:N, :V] = A
    K[N:, :V] = A_pos
    K[N:, V:] = -np.eye(N, dtype=np.float32)
    h = np.concatenate([b.astype(np.float32), np.zeros(N, np.float32)])
    Kn = np.float32(np.sqrt(np.abs(K).sum(0).max() * np.abs(K).sum(1).max()))
    tau = np.float32(0.9) / Kn
    return K, h, tau


def _host_tiles(K, h, tau):
    """Banded constant SBUF images (edge/y permuted) shared by all cores."""
    sigma = tau
    Kx = K[:, :V][np.ix_(Y_PERM, PERM_E)]        # (288, 1012) permuted
    hp = h[Y_PERM]
    tKx = (tau * Kx).astype(np.float32)
    sKx = (sigma * Kx).astype(np.float32)

    # KA1: MM1 rhs band blocks: band0 x[0:360) @0, band1 x[292:512) @360,
    # band1 x[512:720) @580, band2 x[652:720) @788, band2 x[720:1012) @856
    ka1 = np.zeros((128, KA1_W), np.float32)
    for bd, r0, c0, w, off in ((0, 0, 0, 360, 0), (1, 96, 292, 220, 360),
                               (1, 96, 512, 208, 580), (2, 192, 652, 68, 788),
                               (2, 192, 720, 292, 856)):
        ka1[0:96, off:off + w] = tKx[r0:r0 + 96, c0:c0 + w]

    # KS2: MM2 span blocks + h-row at [KS2_H:KS2_H+288) row 0
    ks2 = np.zeros((128, KS2_W), np.float32)
    for ci, (c0, c1, ylo, yhi) in enumerate(MM2_CHUNKS):
        ks2[0:c1 - c0, KS2_OFF[ci]:KS2_OFF[ci + 1]] = sKx[ylo:yhi, c0:c1].T
    ks2[0, KS2_H:KS2_H + YR] = -sigma * hp
    return ka1, ks2


def _per_core_tiles(w_shard, tau):
    """tq for one 64-row batch shard; w_shard (64, 144)."""
    tq = np.zeros((64, F), np.float32)
    tq[:, V:] = tau * w_shard
    return tq


# constsr layout (f32r, read-only): ka1 | ks2
KA1_W = 1148
KS2_H = KS2_OFF[-1]
KS2_W = KS2_H + YR
C_KA1 = 0
C_KS2 = C_KA1 + KA1_W
CR_W = C_KS2 + KS2_W
# constsf layout (f32, read-only): tq | yp0  (yp0 = +sigma*h)
C_TQ = 0
C_YP = C_TQ + F
CF_W = C_YP + YR

FA = 512          # feature split: A = 0:512, B = 512:1156
FB = F - FA       # 644


def _pack_consts(ka1, ks2, tq, yp0):
    cr = np.zeros((128, CR_W), np.float32)
    cr[:, C_KA1:C_KS2] = ka1
    cr[:, C_KS2:CR_W] = ks2
    cf = np.zeros((64, CF_W), np.float32)
    cf[:, C_TQ:C_YP] = tq
    cf[:, C_YP:CF_W] = yp0
    return cr, cf


TAUS = np.zeros(1, np.float32)


def _build_bass():
    from concourse import bass, mybir
    from concourse.tile import TileContext
    from concourse.tile_rust import add_dep_helper
    from concourse.mybir import AluOpType as op

    f32 = mybir.dt.float32
    f32r = mybir.dt.float32r

    nc = bass.Bass()
    d_cr = nc.dram_tensor("constsr", (128, CR_W), f32r, kind="ExternalInput")
    d_cf = nc.dram_tensor("constsf", (64, CF_W), f32, kind="ExternalInput")
    d_z = nc.dram_tensor("z", (64, N), f32, kind="ExternalOutput")

    with TileContext(nc) as tc:
        with (
            tc.tile_pool(name="state", bufs=1) as sp,
            tc.tile_pool(name="psA", bufs=1, space="PSUM") as psA,
            tc.tile_pool(name="psB", bufs=1, space="PSUM") as psB,
            tc.tile_pool(name="psP", bufs=1, space="PSUM") as psP,
            tc.tile_pool(name="psT0", bufs=1, space="PSUM") as psT0,
            tc.tile_pool(name="psT1", bufs=1, space="PSUM") as psT1,
            tc.tile_pool(name="psTY", bufs=1, space="PSUM") as psTY,
        ):
            CONSTR = sp.tile([128, CR_W], f32r)
            CONSTF = sp.tile([64, CF_W], f32)
            KA1 = CONSTR[:, C_KA1:C_KS2]
            KS2 = CONSTR[:, C_KS2:CR_W]
            TQ_A = CONSTF[:, C_TQ:C_TQ + FA]
            TQ_B = CONSTF[:, C_TQ + FA:C_YP]
            U_A = sp.tile([64, FA], f32)
            U_B = sp.tile([64, FB], f32)
            TMP_A = sp.tile([64, FA], f32)
            TMP_B = sp.tile([64, FB], f32)
            PRES_A = sp.tile([64, FA], f32)
            PRES_B = sp.tile([64, FB], f32)
            YP = sp.tile([64, YR], f32)
            YBM = sp.tile([64, YR], f32)
            T3 = sp.tile([64, YR], f32)
            ZPREV = sp.tile([64, N], f32)
            # per-engine scratch tiles (separate so absorber ops never
            # create cross-engine tile deps)
            SCRD = sp.tile([32, 8], f32)
            SCRA = sp.tile([32, 12], f32)
            SCRP = sp.tile([32, 8], f32)
            ONES32 = sp.tile([32, 64], f32)
            ZER128 = sp.tile([128, 192], f32)
            # feat-major u in two wide tiles; ones-row for the h-fold at
            # row LAST_CW of the chunk-9 column block of UFM1
            UFM0 = sp.tile([128, 256], f32r)   # chunks 0..3
            UFM45 = sp.tile([128, 128], f32r)  # chunks 4,5
            UFM67 = sp.tile([128, 128], f32r)  # chunks 6,7
            UFM8 = sp.tile([128, 64], f32r)    # chunk 8
            UFM9 = sp.tile([32, 64], f32r)     # chunk 9 + ones row
            YFM = sp.tile([96, 192], f32r)
            IDENT = sp.tile([128, 128], f32)

            dma1 = nc.sync.dma_start(CONSTR[:, :], d_cr[:, :])
            dma2 = nc.sync.dma_start(CONSTF[:, :], d_cf[:, :])

            pool_insts = [
                nc.gpsimd.memset(IDENT[:, :], 0.0),
                nc.gpsimd.affine_select(
                    out=IDENT[:, :], in_=IDENT[:, :],
                    compare_op=mybir.AluOpType.not_equal, fill=1.0, base=0,
                    pattern=[[-1, 128]], channel_multiplier=1),
            ]
            dve_insts = [
                nc.vector.memset(U_A[:, :], 0.0),
                nc.vector.memset(U_B[:, :], 0.0),
                nc.vector.memset(ONES32[:, :], 1.0),
                nc.vector.memset(ZER128[:, :], 0.0),
                nc.vector.memset(YBM[:, :], 0.0),
            ]

            G_A = psA.tile([64, FA], f32)
            G_B = psB.tile([64, 768], f32)
            P = psP.tile([64, YR], f32)
            TPX = psT0.tile([128, 256], f32)   # chunks 0..3, then 8..9
            TP45 = psT1.tile([128, 128], f32)
            TP67 = psT1.tile([128, 128], f32, tag="tp67")
            TY = psTY.tile([128, 192], f32)

            # This target allows only ONE sem wait per instruction. Tile's
            # wait elision relies on per-engine program order, which the
            # scheduler may permute. So: (a) pin every engine's stream to
            # emission order with no_sync edges, (b) warm each engine with
            # ops that absorb foreign sems one at a time, (c) per iteration,
            # absorber ops pick up semaphores so every real instruction
            # needs at most one new wait.
            prev = {}

            def chain(eng, inst, *sync_deps):
                for d in sync_deps:
                    add_dep_helper(inst.ins, d.ins, True, "warm")
                if eng in prev:
                    add_dep_helper(inst.ins, prev[eng].ins, False, "order")
                prev[eng] = inst
                return inst

            def pe(inst, *d):
                return chain("pe", inst, *d)

            def dve(inst, *d):
                return chain("dve", inst, *d)

            def act(inst, *d):
                return chain("act", inst, *d)

            def pool(inst, *d):
                return chain("pool", inst, *d)

            # engine warmups: absorb one foreign semaphore per instruction
            dve(nc.vector.tensor_copy(SCRD[0:32, 0:4], CONSTF[0:32, 0:4]),
                dma2)
            dve(nc.vector.tensor_scalar_mul(PRES_A[:, :], TQ_A, -1.0))
            dve(nc.vector.tensor_scalar_mul(PRES_B[:, :], TQ_B, -1.0))
            dve(nc.vector.tensor_copy(YP[:, :], CONSTF[:, C_YP:CF_W]))
            pool(nc.gpsimd.tensor_copy(SCRP[0:32, 4:8], CONSTF[0:32, 4:8]),
                 dma2)
            act(nc.scalar.copy(SCRA[0:32, 8:12], IDENT[0:32, 0:4]),
                *pool_insts)
            act(nc.scalar.copy(UFM9[0:32, 0:64], ONES32[:, :]),
                *dve_insts)
            act(nc.scalar.copy(YFM[0:96, 0:192], ZER128[0:96, 0:192]))
            pe(nc.tensor.transpose(G_A[0:64, 0:64], IDENT[0:64, 0:64],
                                   IDENT[0:64, 0:64]),
               *pool_insts)
            pe(nc.tensor.transpose(G_A[0:64, 0:64], U_A[:, 0:64],
                                   IDENT[0:64, 0:64]),
               *dve_insts)
            pe(nc.tensor.matmul(G_A[0:64, 0:64], KS2[0:128, 0:64],
                                KA1[0:128, 0:64], start=True, stop=True))

            for _it in range(ITERS):
                # ACT absorbers: a1 waits on the last ACT op of the previous
                # iteration (the YFM copy); a2 waits on a1's completion.
                act(nc.scalar.copy(SCRA[0:32, 0:4], YFM[0:32, 128:132]))
                act(nc.scalar.copy(SCRA[0:32, 4:8], SCRA[0:32, 0:4]))

                # ---- MM1: G = tau*(y @ Kx), banded segments; closed
                # psum groups per segment (multi-group deps merge on the
                # PE sem for the DVE readers) ----
                from collections import Counter as _C
                segw = _C((c0, c1) for (_, c0, c1, _) in MM1_TAB)
                seen = _C()
                for (bd, c0, c1, first) in MM1_TAB:
                    seen[(c0, c1)] += 1
                    last = seen[(c0, c1)] == segw[(c0, c1)]
                    out = (G_A[:, c0:c1] if c1 <= 512
                           else G_B[:, c0 - 512:c1 - 512])
                    off = _mm1_rhs_off(bd, c0)
                    pe(nc.tensor.matmul(
                        out, YFM[0:96, 64 * bd:64 * bd + 64],
                        KA1[0:96, off:off + (c1 - c0)],
                        start=first, stop=last, skip_group_check=True))

                # ---- u update, A then B (DVE), pres on GPSIMD ----
                dve(nc.vector.tensor_copy(SCRD[0:32, 0:2], PRES_A[0:32, 0:2]))
                dve(nc.vector.scalar_tensor_tensor(
                    TMP_A[:, :], G_A[:, :], -1.0, PRES_A[:, :],
                    op.mult, op.add))
                dve(nc.vector.tensor_scalar(
                    U_A[:, :], TMP_A[:, :], 0.0, 1.0, op.max, op.min))
                dve(nc.vector.tensor_copy(SCRD[0:32, 2:4], PRES_B[0:32, 0:2]))
                dve(nc.vector.scalar_tensor_tensor(
                    TMP_B[:, 0:500], G_B[:, 0:500], -1.0, PRES_B[:, 0:500],
                    op.mult, op.add))
                dve(nc.vector.scalar_tensor_tensor(
                    TMP_B[:, 500:FB], YBM[:, 1::2], float(TAUS[0]),
                    PRES_B[:, 500:FB], op.mult, op.add))
                dve(nc.vector.tensor_scalar(
                    U_B[:, :], TMP_B[:, :], 0.0, 1.0, op.max, op.min))
                pool(nc.gpsimd.tensor_copy(SCRP[0:32, 2:4], SCRP[0:32, 0:2]))
                pool(nc.gpsimd.tensor_sub(PRES_A[:, :], U_A[:, :], TQ_A))
                pool(nc.gpsimd.tensor_sub(PRES_B[:, :], U_B[:, :], TQ_B))
                pool(nc.gpsimd.tensor_copy(SCRP[0:32, 0:2], PRES_B[0:32, 0:2]))

                # ---- transpose u to feat-major; MM2 accumulates P ----
                for c in range(4):        # chunks 0..3 from U_A
                    pe(nc.tensor.transpose(
                        TPX[:, 64 * c:64 * c + 64],
                        U_A[:, 128 * c:128 * c + 128], IDENT[0:64, 0:64]))
                act(nc.scalar.copy(UFM0[:, :], TPX[:, :]))
                # absorber: pick up ufm0's completion so the later TPX
                # read-read serializer deps (chunks 8/9) are pre-covered
                act(nc.scalar.copy(SCRA[0:32, 8:12], UFM0[0:32, 0:4]))
                pe(nc.tensor.matmul(
                    P[:, :], UFM9[0:1, 0:64], KS2[0:1, KS2_H:KS2_H + YR],
                    start=True, stop=True, skip_group_check=True))
                for c in range(4):
                    c0, c1, ylo, yhi = MM2_CHUNKS[c]
                    pe(nc.tensor.matmul(
                        P[:, ylo:yhi], UFM0[0:128, 64 * c:64 * c + 64],
                        KS2[0:128, KS2_OFF[c]:KS2_OFF[c + 1]],
                        start=False, stop=True,
                        skip_group_check=True))
                # chunks 4..9 from U_B, grouped (2 transposes -> copy ->
                # 2 matmuls) so MM2 starts as soon as each pair lands
                def t1(c, dst, col):
                    cw = 128 if c < 9 else LAST_CW
                    pe(nc.tensor.transpose(
                        dst[0:cw, col:col + 64],
                        U_B[:, 128 * (c - 4):128 * (c - 4) + cw],
                        IDENT[0:64, 0:64]))

                def mm2(c, tile, col, stop=False):
                    rows = 128 if c < 7 else 116
                    c0, c1, ylo, yhi = MM2_CHUNKS[c]
                    return pe(nc.tensor.matmul(
                        P[:, ylo:yhi], tile[0:rows, col:col + 64],
                        KS2[0:rows, KS2_OFF[c]:KS2_OFF[c + 1]],
                        start=False, stop=True,
                        skip_group_check=True))

                t1(4, TP45, 0)
                t1(5, TP45, 64)
                t1(6, TP67, 0)
                t1(7, TP67, 64)
                act(nc.scalar.copy(UFM45[:, :], TP45[:, :]))
                act(nc.scalar.copy(UFM67[:, :], TP67[:, :]))
                mm2(4, UFM45, 0)
                mm2(5, UFM45, 64)
                mm2(6, UFM67, 0)
                mm2(7, UFM67, 64)

                # ---- y update: y = relu(YP + 2P);  YP' = y - P ----
                dve(nc.vector.scalar_tensor_tensor(
                    T3[:, :], P[:, :], 2.0, YP[:, :], op.mult, op.add))
                dve(nc.vector.scalar_tensor_tensor(
                    T3[:, 1::2], U_B[:, 500:FB], -2.0 * float(TAUS[0]),
                    T3[:, 1::2], op.mult, op.add))
                # ---- per-band relu -> transpose pipeline ----
                for ci in range(3):
                    dve(nc.vector.tensor_scalar_max(
                        YBM[:, 96 * ci:96 * ci + 96],
                        T3[:, 96 * ci:96 * ci + 96], 0.0))
                    pe(nc.tensor.transpose(
                        TY[0:96, 64 * ci:64 * ci + 64],
                        YBM[:, 96 * ci:96 * ci + 96],
                        IDENT[0:64, 0:64]))
                act(nc.scalar.copy(YFM[0:96, 0:64], TY[0:96, 0:64]))
                act(nc.scalar.copy(YFM[0:96, 64:192], TY[0:96, 64:192]))
                # off-critical: YP for next iter (reads P psum, so DVE)
                dve(nc.vector.scalar_tensor_tensor(
                    YP[:, :], P[:, :], -1.0, YBM[:, :], op.mult, op.add))
                dve(nc.vector.scalar_tensor_tensor(
                    YP[:, 1::2], U_B[:, 500:FB], float(TAUS[0]),
                    YP[:, 1::2], op.mult, op.add))


            zdma = nc.sync.dma_start(d_z[:, :], U_B[:, FB - N:FB])
            # tail fence: the framework drain waits on every proc, but the
            # ISA allows one wait per instruction — absorb them one at a
            # time with SP nops so the drain's own waits are elided.
            for d in (dma1, dma2, prev["pool"], prev["act"], prev["pe"],
                      prev["dve"], zdma):
                nn = nc.sync.nop()
                add_dep_helper(nn.ins, d.ins, True, "tail fence")
    return nc


LAST_RESULT = None


def kernel(weights, A, A_pos, b, _trace=False):
    weights = np.asarray(weights, np.float32)
    A = np.asarray(A, np.float32)
    A_pos = np.asarray(A_pos, np.float32)
    b = np.asarray(b, np.float32)

    K, h, tau = _build_constants(A, A_pos, b)
    TAUS[0] = tau
    ka1, ks2 = _host_tiles(K, h, tau)
    yp0 = np.broadcast_to((tau * h)[Y_PERM], (64, YR)).astype(
        np.float32).copy()

    nc = _build_bass()

    in_maps = []
    for core in range(N_CORES):
        w_shard = weights[core * BS:(core + 1) * BS].reshape(BS, N)
        tq = _per_core_tiles(w_shard, tau)
        cr, cf = _pack_consts(ka1, ks2, tq, yp0)
        in_maps.append({"constsr": cr, "constsf": cf})

    from concourse.bass_utils import run_bass_kernel_spmd
    res = run_bass_kernel_spmd(nc, in_maps, core_ids=list(range(N_CORES)),
                               trace=_trace)
    global LAST_RESULT
    LAST_RESULT = res
    outs = [np.asarray(res.results[c]["z"]) for c in range(N_CORES)]
    z = np.concatenate(outs, axis=0).reshape(B, N_GRID, N_GRID)
    return z.astype(np.float32)


if __name__ == "__main__":
    TAUS[0] = 0.1
    rng = np.random.default_rng(0)
    w = rng.random((B, N_GRID, N_GRID), np.float32)
    # smoke build only
    _build_bass()
    print("bass build OK")



# revision 37
# speedup vs baseline: 1.1346x; 1.0363x over previous
"""Trainium2 Bass kernel for nn_CvxDifflayer (batched PDHG LP solver).

Math (per batch row b):
    u_{k+1} = clip(u_k - tau*(q + y_k @ K), 0, 1)
    ubar    = 2*u_{k+1} - u_k
    y_{k+1} = relu(y_k + sigma*(ubar @ K.T - h))
    out z   = u_300[:, V:]  reshaped (12, 12)

Device reformulation (exact, per 64-batch shard):
    G_k  = tau*q + tau*(y_k @ K)        MM1: lhsT = Y feat-major, rhs = tau*K
    u    = clip(pres_k - G'_k)          where pres_k = u_k - tau*q, G' = tau*(yK)
    pres = u - tau*q                    (GPSIMD, off critical path)
    P_k  = sigma*(u @ K.T) - sigma*h    MM2: lhsT = u feat-major (PE transposes),
                                        h folded via constant ones-row in lhsT
    y    = relu(y + 2*P_k - P_{k-1})    (P_{-1} = -sigma*h since u_0 = 0)

Layouts per core (batch shard Bs=64):
    U     [128, 578]  batch-major fold: row 64*hf+b, col j = feature 578*hf+j
    UFM   [128, 640]  feat-major: chunk c cols [128c:128c+128); within chunk,
                      col 64*h+b = batch b of fold-half h; rows = fold-col
                      128c+i; chunk 4 rows 0:66 (+ row 66 = ones for h-fold)
    YBM   [64, 288]   batch-major y
    YFM   [128, 192]  feat-major y: block ci cols [64ci:64ci+64) = batch,
                      rows = y-row 128ci+i
"""

import sys

for _p in ("/opt/trn_rl_repo", "/opt/pypackages"):
    if _p not in sys.path:
        sys.path.insert(0, _p)

import numpy as np

N_GRID = 12
N = 144          # nodes
V = 1012         # directed edges
F = V + N        # 1156 primal vars
YR = 2 * N       # 288 dual vars
B = 512
BS = 64          # batch per core
N_CORES = 8
ITERS = 300
FPAD = 1280      # MM1 free dim padded so all chunks >= 256 (fp32r rate)
FM_CHUNKS = 10   # ceil(1156/128)
LAST_CW = F - 9 * 128      # 4


def _tables():
    offs = [(p, q) for p in (-1, 0, 1) for q in (-1, 0, 1) if (p, q) != (0, 0)]
    es = []
    for i in range(N_GRID):
        for j in range(N_GRID):
            for p, q in offs:
                ii, jj = i + p, j + q
                if 0 <= ii < N_GRID and 0 <= jj < N_GRID:
                    es.append((i * N_GRID + j, ii * N_GRID + jj))

    def nb(node):
        return (node // N_GRID) // 4

    keys = [(min(nb(s), nb(t)), max(nb(s), nb(t)), s) for (s, t) in es]
    order = sorted(range(V), key=lambda e: keys[e])
    perm_e = np.array(order, dtype=np.int64)
    segs = []
    st, cur = 0, keys[order[0]][:2]
    for j in range(1, V + 1):
        if j == V or keys[order[j]][:2] != cur:
            segs.append((cur[0], cur[1], st, j))
            if j < V:
                cur, st = keys[order[j]][:2], j
    ss = []
    for (b1, b2, s0, s1) in segs:
        if s0 < 512 < s1:
            ss += [(b1, b2, s0, 512), (b1, b2, 512, s1)]
        else:
            ss.append((b1, b2, s0, s1))
    mm1 = []
    for bank in (0, 1):
        for bd in range(3):
            for (b1, b2, s0, s1) in ss:
                if bd in (b1, b2) and ((s1 <= 512) == (bank == 0)):
                    mm1.append((bd, s0, s1, bd == b1))
    y_perm = np.empty(YR, dtype=np.int64)
    y_perm[0::2] = np.arange(N)
    y_perm[1::2] = N + np.arange(N)
    chunks = []
    for c0 in range(0, V, 128):
        c1 = min(c0 + 128, V)
        nodes = set()
        for j in range(c0, c1):
            s, t = es[perm_e[j]]
            nodes.add(s)
            nodes.add(t)
        chunks.append((c0, c1, 2 * min(nodes), 2 * max(nodes) + 2))
    return perm_e, y_perm, mm1, chunks


PERM_E, Y_PERM, MM1_TAB, MM2_CHUNKS = _tables()
BAND_COL0 = [0, 292, 652]
KA1_OFF = [0, 360, 788]
KS2_OFF = np.cumsum(
    [0] + [hi - lo for (_, _, lo, hi) in MM2_CHUNKS]).tolist()


def _mm1_rhs_off(bd, c0):
    if bd == 1 and c0 >= 512:
        return 580 + (c0 - 512)
    if bd == 2 and c0 >= 720:
        return 856 + (c0 - 720)
    return KA1_OFF[bd] + (c0 - BAND_COL0[bd])


def _build_constants(A, A_pos, b):
    K = np.zeros((YR, F), np.float32)
    K[:N, :V] = A
    K[N:, :V] = A_pos
    K[N:, V:] = -np.eye(N, dtype=np.float32)
    h = np.concatenate([b.astype(np.float32), np.zeros(N, np.float32)])
    Kn = np.float32(np.sqrt(np.abs(K).sum(0).max() * np.abs(K).sum(1).max()))
    tau = np.float32(0.9) / Kn
    return K, h, tau


def _host_tiles(K, h, tau):
    """Banded constant SBUF images (edge/y permuted) shared by all cores."""
    sigma = tau
    Kx = K[:, :V][np.ix_(Y_PERM, PERM_E)]        # (288, 1012) permuted
    hp = h[Y_PERM]
    tKx = (tau * Kx).astype(np.float32)
    sKx = (sigma * Kx).astype(np.float32)

    # KA1: MM1 rhs band blocks: band0 x[0:360) @0, band1 x[292:512) @360,
    # band1 x[512:720) @580, band2 x[652:720) @788, band2 x[720:1012) @856
    ka1 = np.zeros((128, KA1_W), np.float32)
    for bd, r0, c0, w, off in ((0, 0, 0, 360, 0), (1, 96, 292, 220, 360),
                               (1, 96, 512, 208, 580), (2, 192, 652, 68, 788),
                               (2, 192, 720, 292, 856)):
        ka1[0:96, off:off + w] = tKx[r0:r0 + 96, c0:c0 + w]

    # KS2: MM2 span blocks + h-row at [KS2_H:KS2_H+288) row 0
    ks2 = np.zeros((128, KS2_W), np.float32)
    for ci, (c0, c1, ylo, yhi) in enumerate(MM2_CHUNKS):
        ks2[0:c1 - c0, KS2_OFF[ci]:KS2_OFF[ci + 1]] = sKx[ylo:yhi, c0:c1].T
    ks2[0, KS2_H:KS2_H + YR] = -sigma * hp
    return ka1, ks2


def _per_core_tiles(w_shard, tau):
    """tq for one 64-row batch shard; w_shard (64, 144)."""
    tq = np.zeros((64, F), np.float32)
    tq[:, V:] = tau * w_shard
    return tq


# constsr layout (f32r, read-only): ka1 | ks2
KA1_W = 1148
KS2_H = KS2_OFF[-1]
KS2_W = KS2_H + YR
C_KA1 = 0
C_KS2 = C_KA1 + KA1_W
CR_W = C_KS2 + KS2_W
# constsf layout (f32, read-only): tq | yp0  (yp0 = +sigma*h)
C_TQ = 0
C_YP = C_TQ + F
CF_W = C_YP + YR

FA = 512          # feature split: A = 0:512, B = 512:1156
FB = F - FA       # 644


def _pack_consts(ka1, ks2, tq, yp0):
    cr = np.zeros((128, CR_W), np.float32)
    cr[:, C_KA1:C_KS2] = ka1
    cr[:, C_KS2:CR_W] = ks2
    cf = np.zeros((64, CF_W), np.float32)
    cf[:, C_TQ:C_YP] = tq
    cf[:, C_YP:CF_W] = yp0
    return cr, cf


TAUS = np.zeros(1, np.float32)


def _build_bass():
    from concourse import bass, mybir
    from concourse.tile import TileContext
    from concourse.tile_rust import add_dep_helper
    from concourse.mybir import AluOpType as op

    f32 = mybir.dt.float32
    f32r = mybir.dt.float32r

    nc = bass.Bass()
    d_cr = nc.dram_tensor("constsr", (128, CR_W), f32r, kind="ExternalInput")
    d_cf = nc.dram_tensor("constsf", (64, CF_W), f32, kind="ExternalInput")
    d_z = nc.dram_tensor("z", (64, N), f32, kind="ExternalOutput")

    with TileContext(nc) as tc:
        with (
            tc.tile_pool(name="state", bufs=1) as sp,
            tc.tile_pool(name="psA", bufs=1, space="PSUM") as psA,
            tc.tile_pool(name="psB", bufs=1, space="PSUM") as psB,
            tc.tile_pool(name="psP", bufs=1, space="PSUM") as psP,
            tc.tile_pool(name="psT0", bufs=1, space="PSUM") as psT0,
            tc.tile_pool(name="psT1", bufs=1, space="PSUM") as psT1,
            tc.tile_pool(name="psTY", bufs=1, space="PSUM") as psTY,
        ):
            CONSTR = sp.tile([128, CR_W], f32r)
            CONSTF = sp.tile([64, CF_W], f32)
            KA1 = CONSTR[:, C_KA1:C_KS2]
            KS2 = CONSTR[:, C_KS2:CR_W]
            TQ_A = CONSTF[:, C_TQ:C_TQ + FA]
            TQ_B = CONSTF[:, C_TQ + FA:C_YP]
            U_A = sp.tile([64, FA], f32)
            U_B = sp.tile([64, FB], f32)
            TMP_A = sp.tile([64, FA], f32)
            TMP_B = sp.tile([64, FB], f32)
            PRES_A = sp.tile([64, FA], f32)
            PRES_B = sp.tile([64, FB], f32)
            YP = sp.tile([64, YR], f32)
            YBM = sp.tile([64, YR], f32)
            T3 = sp.tile([64, YR], f32)
            ZPREV = sp.tile([64, N], f32)
            # per-engine scratch tiles (separate so absorber ops never
            # create cross-engine tile deps)
            SCRD = sp.tile([32, 8], f32)
            SCRA = sp.tile([32, 12], f32)
            SCRP = sp.tile([32, 8], f32)
            ONES32 = sp.tile([32, 64], f32)
            ZER128 = sp.tile([128, 192], f32)
            # feat-major u in two wide tiles; ones-row for the h-fold at
            # row LAST_CW of the chunk-9 column block of UFM1
            UFM0 = sp.tile([128, 256], f32r)   # chunks 0..3
            UFM45 = sp.tile([128, 128], f32r)  # chunks 4,5
            UFM67 = sp.tile([128, 128], f32r)  # chunks 6,7
            UFM8 = sp.tile([128, 64], f32r)    # chunk 8
            UFM9 = sp.tile([32, 64], f32r)     # chunk 9 + ones row
            YFM = sp.tile([96, 192], f32r)
            IDENT = sp.tile([128, 128], f32)

            dma1 = nc.sync.dma_start(CONSTR[:, :], d_cr[:, :])
            dma2 = nc.sync.dma_start(CONSTF[:, :], d_cf[:, :])

            pool_insts = [
                nc.gpsimd.memset(IDENT[:, :], 0.0),
                nc.gpsimd.affine_select(
                    out=IDENT[:, :], in_=IDENT[:, :],
                    compare_op=mybir.AluOpType.not_equal, fill=1.0, base=0,
                    pattern=[[-1, 128]], channel_multiplier=1),
            ]
            dve_insts = [
                nc.vector.memset(U_A[:, :], 0.0),
                nc.vector.memset(U_B[:, :], 0.0),
                nc.vector.memset(ONES32[:, :], 1.0),
                nc.vector.memset(ZER128[:, :], 0.0),
                nc.vector.memset(YBM[:, :], 0.0),
            ]

            G_A = psA.tile([64, FA], f32)
            G_B = psB.tile([64, 768], f32)
            P = psP.tile([64, YR], f32)
            TPX = psT0.tile([128, 256], f32)   # chunks 0..3, then 8..9
            TP45 = psT1.tile([128, 128], f32)
            TP67 = psT1.tile([128, 128], f32, tag="tp67")
            TY = psTY.tile([128, 192], f32)

            # This target allows only ONE sem wait per instruction. Tile's
            # wait elision relies on per-engine program order, which the
            # scheduler may permute. So: (a) pin every engine's stream to
            # emission order with no_sync edges, (b) warm each engine with
            # ops that absorb foreign sems one at a time, (c) per iteration,
            # absorber ops pick up semaphores so every real instruction
            # needs at most one new wait.
            prev = {}

            def chain(eng, inst, *sync_deps):
                for d in sync_deps:
                    add_dep_helper(inst.ins, d.ins, True, "warm")
                if eng in prev:
                    add_dep_helper(inst.ins, prev[eng].ins, False, "order")
                prev[eng] = inst
                return inst

            def pe(inst, *d):
                return chain("pe", inst, *d)

            def dve(inst, *d):
                return chain("dve", inst, *d)

            def act(inst, *d):
                return chain("act", inst, *d)

            def pool(inst, *d):
                return chain("pool", inst, *d)

            # engine warmups: absorb one foreign semaphore per instruction
            dve(nc.vector.tensor_copy(SCRD[0:32, 0:4], CONSTF[0:32, 0:4]),
                dma2)
            dve(nc.vector.tensor_scalar_mul(PRES_A[:, :], TQ_A, -1.0))
            dve(nc.vector.tensor_scalar_mul(PRES_B[:, :], TQ_B, -1.0))
            dve(nc.vector.tensor_copy(YP[:, :], CONSTF[:, C_YP:CF_W]))
            pool(nc.gpsimd.tensor_copy(SCRP[0:32, 4:8], CONSTF[0:32, 4:8]),
                 dma2)
            act(nc.scalar.copy(SCRA[0:32, 8:12], IDENT[0:32, 0:4]),
                *pool_insts)
            act(nc.scalar.copy(UFM9[0:32, 0:64], ONES32[:, :]),
                *dve_insts)
            act(nc.scalar.copy(YFM[0:96, 0:192], ZER128[0:96, 0:192]))
            pe(nc.tensor.transpose(G_A[0:64, 0:64], IDENT[0:64, 0:64],
                                   IDENT[0:64, 0:64]),
               *pool_insts)
            pe(nc.tensor.transpose(G_A[0:64, 0:64], U_A[:, 0:64],
                                   IDENT[0:64, 0:64]),
               *dve_insts)
            pe(nc.tensor.matmul(G_A[0:64, 0:64], KS2[0:128, 0:64],
                                KA1[0:128, 0:64], start=True, stop=True))

            for _it in range(ITERS):
                # ACT absorbers: a1 waits on the last ACT op of the previous
                # iteration (the YFM copy); a2 waits on a1's completion.
                act(nc.scalar.copy(SCRA[0:32, 0:4], YFM[0:32, 128:132]))
                act(nc.scalar.copy(SCRA[0:32, 4:8], SCRA[0:32, 0:4]))

                # ---- MM1: G = tau*(y @ Kx), banded segments; closed
                # psum groups per segment (multi-group deps merge on the
                # PE sem for the DVE readers) ----
                from collections import Counter as _C
                segw = _C((c0, c1) for (_, c0, c1, _) in MM1_TAB)
                seen = _C()
                for (bd, c0, c1, first) in MM1_TAB:
                    seen[(c0, c1)] += 1
                    last = seen[(c0, c1)] == segw[(c0, c1)]
                    out = (G_A[:, c0:c1] if c1 <= 512
                           else G_B[:, c0 - 512:c1 - 512])
                    off = _mm1_rhs_off(bd, c0)
                    pe(nc.tensor.matmul(
                        out, YFM[0:96, 64 * bd:64 * bd + 64],
                        KA1[0:96, off:off + (c1 - c0)],
                        start=first, stop=last, skip_group_check=True))

                # ---- u update, A then B (DVE), pres on GPSIMD ----
                dve(nc.vector.tensor_copy(SCRD[0:32, 0:2], PRES_A[0:32, 0:2]))
                dve(nc.vector.scalar_tensor_tensor(
                    TMP_A[:, :], G_A[:, :], -1.0, PRES_A[:, :],
                    op.mult, op.add))
                dve(nc.vector.tensor_scalar(
                    U_A[:, :], TMP_A[:, :], 0.0, 1.0, op.max, op.min))
                dve(nc.vector.tensor_copy(SCRD[0:32, 2:4], PRES_B[0:32, 0:2]))
                dve(nc.vector.scalar_tensor_tensor(
                    TMP_B[:, 0:500], G_B[:, 0:500], -1.0, PRES_B[:, 0:500],
                    op.mult, op.add))
                dve(nc.vector.scalar_tensor_tensor(
                    TMP_B[:, 500:FB], YBM[:, 1::2], float(TAUS[0]),
                    PRES_B[:, 500:FB], op.mult, op.add))
                dve(nc.vector.tensor_scalar(
                    U_B[:, :], TMP_B[:, :], 0.0, 1.0, op.max, op.min))
                dve(nc.vector.scalar_tensor_tensor(
                    YP[:, 1::2], U_B[:, 500:FB], -2.0 * float(TAUS[0]),
                    YP[:, 1::2], op.mult, op.add))
                pool(nc.gpsimd.tensor_copy(SCRP[0:32, 2:4], SCRP[0:32, 0:2]))
                pool(nc.gpsimd.tensor_sub(PRES_A[:, :], U_A[:, :], TQ_A))
                pool(nc.gpsimd.tensor_sub(PRES_B[:, :], U_B[:, :], TQ_B))
                pool(nc.gpsimd.tensor_copy(SCRP[0:32, 0:2], PRES_B[0:32, 0:2]))

                # ---- transpose u to feat-major; MM2 accumulates P ----
                for c in range(4):        # chunks 0..3 from U_A
                    pe(nc.tensor.transpose(
                        TPX[:, 64 * c:64 * c + 64],
                        U_A[:, 128 * c:128 * c + 128], IDENT[0:64, 0:64]))
                act(nc.scalar.copy(UFM0[:, :], TPX[:, :]))
                # absorber: pick up ufm0's completion so the later TPX
                # read-read serializer deps (chunks 8/9) are pre-covered
                act(nc.scalar.copy(SCRA[0:32, 8:12], UFM0[0:32, 0:4]))
                pe(nc.tensor.matmul(
                    P[:, :], UFM9[0:1, 0:64], KS2[0:1, KS2_H:KS2_H + YR],
                    start=True, stop=True, skip_group_check=True))
                for c in range(4):
                    c0, c1, ylo, yhi = MM2_CHUNKS[c]
                    pe(nc.tensor.matmul(
                        P[:, ylo:yhi], UFM0[0:128, 64 * c:64 * c + 64],
                        KS2[0:128, KS2_OFF[c]:KS2_OFF[c + 1]],
                        start=False, stop=True,
                        skip_group_check=True))
                # chunks 4..9 from U_B, grouped (2 transposes -> copy ->
                # 2 matmuls) so MM2 starts as soon as each pair lands
                def t1(c, dst, col):
                    cw = 128 if c < 9 else LAST_CW
                    pe(nc.tensor.transpose(
                        dst[0:cw, col:col + 64],
                        U_B[:, 128 * (c - 4):128 * (c - 4) + cw],
                        IDENT[0:64, 0:64]))

                def mm2(c, tile, col, stop=False):
                    rows = 128 if c < 7 else 116
                    c0, c1, ylo, yhi = MM2_CHUNKS[c]
                    return pe(nc.tensor.matmul(
                        P[:, ylo:yhi], tile[0:rows, col:col + 64],
                        KS2[0:rows, KS2_OFF[c]:KS2_OFF[c + 1]],
                        start=False, stop=True,
                        skip_group_check=True))

                t1(4, TP45, 0)
                t1(5, TP45, 64)
                t1(6, TP67, 0)
                t1(7, TP67, 64)
                act(nc.scalar.copy(UFM45[:, :], TP45[:, :]))
                act(nc.scalar.copy(UFM67[:, :], TP67[:, :]))
                mm2(4, UFM45, 0)
                mm2(5, UFM45, 64)
                mm2(6, UFM67, 0)
                last_mm2 = mm2(7, UFM67, 64)

                # ---- y update: y = relu(YP + 2P); the -2*sigma*z fix
                # was pre-folded into YP above. Absorber carries the PE
                # wait so T3 keeps only its own-sem wait.
                _ab = dve(nc.vector.tensor_copy(SCRD[0:32, 4:6],
                                                SCRD[0:32, 6:8]))
                add_dep_helper(_ab.ins, last_mm2.ins, True, "absorb")
                dve(nc.vector.scalar_tensor_tensor(
                    T3[:, :], P[:, :], 2.0, YP[:, :], op.mult, op.add))
                # ---- per-band relu -> transpose pipeline ----
                for ci in range(3):
                    dve(nc.vector.tensor_scalar_max(
                        YBM[:, 96 * ci:96 * ci + 96],
                        T3[:, 96 * ci:96 * ci + 96], 0.0))
                    pe(nc.tensor.transpose(
                        TY[0:96, 64 * ci:64 * ci + 64],
                        YBM[:, 96 * ci:96 * ci + 96],
                        IDENT[0:64, 0:64]))
                act(nc.scalar.copy(YFM[0:96, 0:64], TY[0:96, 0:64]))
                act(nc.scalar.copy(YFM[0:96, 64:192], TY[0:96, 64:192]))
                # off-critical: YP for next iter (reads P psum, so DVE)
                dve(nc.vector.scalar_tensor_tensor(
                    YP[:, :], P[:, :], -1.0, YBM[:, :], op.mult, op.add))
                dve(nc.vector.scalar_tensor_tensor(
                    YP[:, 1::2], U_B[:, 500:FB], float(TAUS[0]),
                    YP[:, 1::2], op.mult, op.add))


            zdma = nc.sync.dma_start(d_z[:, :], U_B[:, FB - N:FB])
            # tail fence: the framework drain waits on every proc, but the
            # ISA allows one wait per instruction — absorb them one at a
            # time with SP nops so the drain's own waits are elided.
            for d in (dma1, dma2, prev["pool"], prev["act"], prev["pe"],
                      prev["dve"], zdma):
                nn = nc.sync.nop()
                add_dep_helper(nn.ins, d.ins, True, "tail fence")
    return nc


LAST_RESULT = None


def kernel(weights, A, A_pos, b, _trace=False):
    weights = np.asarray(weights, np.float32)
    A = np.asarray(A, np.float32)
    A_pos = np.asarray(A_pos, np.float32)
    b = np.asarray(b, np.float32)

    K, h, tau = _build_constants(A, A_pos, b)
    TAUS[0] = tau
    ka1, ks2 = _host_tiles(K, h, tau)
    yp0 = np.broadcast_to((tau * h)[Y_PERM], (64, YR)).astype(
        np.float32).copy()

    nc = _build_bass()

    in_maps = []
    for core in range(N_CORES):
        w_shard = weights[core * BS:(core + 1) * BS].reshape(BS, N)
        tq = _per_core_tiles(w_shard, tau)
        cr, cf = _pack_consts(ka1, ks2, tq, yp0)
        in_maps.append({"constsr": cr, "constsf": cf})

    from concourse.bass_utils import run_bass_kernel_spmd
    res = run_bass_kernel_spmd(nc, in_maps, core_ids=list(range(N_CORES)),
                               trace=_trace)
    global LAST_RESULT
    LAST_RESULT = res
    outs = [np.asarray(res.results[c]["z"]) for c in range(N_CORES)]
    z = np.concatenate(outs, axis=0).reshape(B, N_GRID, N_GRID)
    return z.astype(np.float32)


if __name__ == "__main__":
    TAUS[0] = 0.1
    rng = np.random.default_rng(0)
    w = rng.random((B, N_GRID, N_GRID), np.float32)
    # smoke build only
    _build_bass()
    print("bass build OK")

